# revision 14
# baseline (speedup 1.0000x reference)
"""Expert-parallel MoE FFN kernel for Trainium2 (8 NeuronCores).

Problem: inputs [B=2, E=8, C=8192, H=512], per-expert FFN
    h   = gelu_tanh(x_e @ w1_e + b1_e)        (w1: [E, H, F=2048])
    out = h @ w2_e + b2_e                     (w2: [E, F, H])

Sharding: expert-parallel — core e owns expert e's tokens [B*C, H] and
weights; no cross-core communication.

Per-core dataflow (matmuls in float32r = full-rate fp32 on the PE):
    x is transposed on the HOST to xT [H, tok] so each macro tile loads
    straight into the GEMM1 rhs layout — no PE transposes at all.
    GEMM1: hT[f,:] += w1[hk, f-chunk].T @ xT[hk, :]   (lhsT = w1, natural layout)
    gelu on ACT during PSUM->SBUF eviction (bias = b1 per-partition)
    GEMM2: out[tok,:] += hT[fk, tok-chunk].T @ w2[fk, :] (lhsT = hT, rhs = w2 natural)
    b2 add on DVE during PSUM->SBUF eviction

The x loads are prefetched two macro tiles ahead on the sync (HWDGE)
queue; output stores go out on the gpsimd (SWDGE) queue so an input
load is never queued behind an output store.

FP32r discipline: the BIR verifier requires every producer of an f32r
matmul input to round its output to f32r. xT and the weights are DMA'd
as raw bytes into f32r tiles; hT is produced by ACT gelu writing into
an f32r tile (rounding happens on the write).
"""

import numpy as np

_B, _E, _C, _H, _F = 2, 8, 8192, 512, 2048
_TOK = _B * _C  # 16384 tokens per expert
_P = 128
_T = 512  # tokens per macro tile

# "bf16" halves nothing on paper (cost model says 1 cyc/row either way)
# but measures materially faster on HW thanks to FWL weight loads; the
# end-to-end rel err vs the fp32 reference is ~3.4e-3 (numpy estimate),
# well inside the 2e-2 gate.
_MM_MODE = "bf16"  # "bf16" | "f32r"

_cache = {}


def build_nc(tok=_TOK, act_name="Gelu_apprx_tanh", n_devices=_E,
             loop_reps=1, skip=(), psum_cfg=(6, 2), prefetch=2,
             out_q="gpsimd", mm_mode=_MM_MODE, unroll=1):
    import contextlib

    import concourse.mybir as mybir
    import concourse.tile as tile
    from concourse import bacc

    H, F, P, T = _H, _F, _P, _T
    HK, FK = H // P, F // P  # 4, 16 contraction chunks
    NJ = T // P  # 4 token sub-blocks per macro tile
    NM = tok // T  # macro tiles
    f32 = mybir.dt.float32
    f32r = mybir.dt.float32r
    mmdt = mybir.dt.bfloat16 if mm_mode == "bf16" else f32r
    act = getattr(mybir.ActivationFunctionType, act_name)

    nc = bacc.Bacc("TRN2", debug=False, target_bir_lowering=False,
                   num_devices=n_devices)
    # x arrives pre-transposed (and pre-cast in bf16 mode) from the host
    x = nc.dram_tensor("x", [H, tok], mmdt, kind="ExternalInput").ap()
    w1 = nc.dram_tensor("w1", [H, F], mmdt, kind="ExternalInput").ap()
    b1 = nc.dram_tensor("b1", [F], f32, kind="ExternalInput").ap()
    w2 = nc.dram_tensor("w2", [F, H], mmdt, kind="ExternalInput").ap()
    b2 = nc.dram_tensor("b2", [H], f32, kind="ExternalInput").ap()
    out = nc.dram_tensor("out", [tok, H], f32, kind="ExternalOutput").ap()

    with tile.TileContext(nc) as tc:
        with (
            tc.tile_pool(name="const", bufs=1) as const,
            tc.tile_pool(name="xt", bufs=1 + prefetch) as xt_pool,
            tc.tile_pool(name="ht", bufs=2) as ht_pool,
            tc.tile_pool(name="obuf", bufs=2) as o_pool,
            tc.tile_pool(name="scr", bufs=4) as scratch,
            tc.tile_pool(name="ps1", bufs=psum_cfg[0], space="PSUM") as ps1,
            tc.tile_pool(name="ps2", bufs=psum_cfg[1], space="PSUM") as ps2,
        ):
            # --- weights / constants, resident in SBUF for the whole kernel.
            # Critical path to the first GEMM1 matmul is w1's fm=0 slice +
            # xt(0); only that slice rides the sync queue ahead of the x
            # prefetches. Everything else (b-vectors, the rest of w1, w2)
            # rides the scalar engine's HWDGE ring, which GEMM1 overtakes
            # only after several macro tiles.
            w1_sb = const.tile([P, HK, F], mmdt)
            w2_sb = const.tile([P, FK, H], mmdt)
            nc.sync.dma_start(
                w1_sb[:, :, 0:P],
                w1[:, 0:P].rearrange("(hk p) f -> p hk f", p=P))
            b1_sb = const.tile([P, FK], f32)
            nc.scalar.dma_start(b1_sb[:], b1.rearrange("(fk p) -> p fk", p=P))
            b2_row = const.tile([1, H], f32)
            nc.scalar.dma_start(b2_row[:], b2[None, :])
            WC = (F - P) // 3
            for c in range(3):
                fs = P + c * WC
                nc.scalar.dma_start(
                    w1_sb[:, :, fs:fs + WC],
                    w1[:, fs:fs + WC].rearrange("(hk p) f -> p hk f", p=P))
            nc.scalar.dma_start(
                w2_sb[:], w2.rearrange("(fk p) h -> p fk h", p=P))
            ones = const.tile([1, P], f32)
            nc.any.memset(ones[:], 1.0)
            # ~40 dummy matmuls warm the PE HAM clock gate (cold = 1.2 GHz,
            # warm = 2.4 GHz after ~3.4us of activity) while the head DMAs
            # are still in flight, so the real GEMM stream starts warm.
            ps_warm = ps1.tile([P, T], f32, tag="ph", name="warm")
            for wi in range(40):
                nc.tensor.matmul(ps_warm[:, 0:P], ones[:], ones[:],
                                 start=(wi == 0), stop=(wi == 39))
            # broadcast b2 across all 128 partitions via a K=1 matmul
            b2_bc = const.tile([P, H], f32)
            ps_b2 = ps2.tile([P, H], f32, tag="po")
            nc.tensor.matmul(ps_b2[:], ones[:], b2_row[:], start=True, stop=True)
            nc.vector.tensor_copy(b2_bc[:], ps_b2[:])

            def load_x(key, m):
                """Prefetch macro tile m of xT into SBUF (GEMM1 rhs layout)."""
                xt = xt_pool.tile([P, HK, T], mmdt, name=f"xt{key}", tag="xt")
                if "indma" not in skip:
                    nc.sync.dma_start(
                        xt[:],
                        x[:, m * T:(m + 1) * T].rearrange(
                            "(hk p) t -> p hk t", p=P))
                return xt

            def g1_group(xt, ht, m, fm):
                """One GEMM1 accumulation group (4 matmuls) + gelu eviction.

                Evictions alternate engines: even fm goes ACT gelu straight
                from PSUM; odd fm is DVE-copied to SBUF scratch first and
                gelu'd from there. Back-to-back ACT PSUM evictions of
                consecutive groups collapse PE throughput ~17x (measured);
                alternating eliminates that.
                """
                ph = ps1.tile([P, T], f32, name=f"ph{m}_{fm}", tag="ph")
                for hk in range(HK):
                    nc.tensor.matmul(
                        ph[:],
                        w1_sb[:, hk, fm * P:(fm + 1) * P],
                        xt[:, hk, :],
                        start=(hk == 0),
                        stop=(hk == HK - 1),
                    )
                if fm % 2 == 0:
                    nc.scalar.activation(
                        ht[:, fm, :], ph[:], act, bias=b1_sb[:, fm:fm + 1])
                else:
                    sc = scratch.tile([P, T], f32, name=f"sc{m}_{fm}", tag="sc")
                    nc.vector.tensor_copy(sc[:], ph[:])
                    nc.scalar.activation(
                        ht[:, fm, :], sc[:], act, bias=b1_sb[:, fm:fm + 1])

            def g2_group(ht, obig, j):
                """One GEMM2 accumulation group (16 matmuls) + b2 eviction."""
                po = ps2.tile([P, H], f32, tag="po", name="po")
                for fk in range(FK):
                    nc.tensor.matmul(
                        po[:],
                        ht[:, fk, j * P:(j + 1) * P],
                        w2_sb[:, fk, :],
                        start=(fk == 0),
                        stop=(fk == FK - 1),
                    )
                nc.vector.tensor_add(obig[:, j, :], po[:], b2_bc[:])

            def new_ht(key):
                return ht_pool.tile([P, FK, T], mmdt,
                                    name=f"ht{key}", tag="ht")

            out_dma = {"gpsimd": nc.gpsimd, "sync": nc.sync,
                       "scalar": nc.scalar}[out_q]

            # loop_reps > 1 wraps the body in a hardware loop — used only by
            # the timing harness to amortize per-dispatch overhead. For_i
            # inserts an all-engine barrier per iteration, so `unroll`
            # repeats the body within one iteration to keep the measured
            # slope closer to the barrier-free steady state.
            assert loop_reps % unroll == 0
            n_iters = loop_reps // unroll
            reps_ctx = (tc.For_i(0, n_iters, 1) if n_iters > 1
                        else contextlib.nullcontext())
            # Software pipeline, one macro ahead on GEMM1, `prefetch` macros
            # ahead on the x DMA:
            #   [dma x(m+2)] ; G1(m+1) x16 ; G2(m) x4 ; out(m)
            # GEMM1(m+1) runs before GEMM2(m) so ACT gelu evictions of ht(m+1)
            # complete under GEMM2(m)'s PE window.
            do_g1 = "gemm1" not in skip
            do_g2 = "gemm2" not in skip

            def emit_body(rep):
                xts = {m: load_x(f"{rep}_{m}", m)
                       for m in range(min(prefetch, NM))}
                ht = new_ht(f"{rep}_0")
                if do_g1:
                    for fm in range(FK):
                        g1_group(xts[0], ht, f"{rep}_0", fm)
                for m in range(NM):
                    r = m * T
                    if m + prefetch < NM:
                        xts[m + prefetch] = load_x(
                            f"{rep}_{m + prefetch}", m + prefetch)
                    if m + 1 < NM:
                        ht_next = new_ht(f"{rep}_{m + 1}")
                        if do_g1:
                            for fm in range(FK):
                                g1_group(xts[m + 1], ht_next,
                                         f"{rep}_{m + 1}", fm)
                    else:
                        ht_next = None
                    xts.pop(m, None)
                    obig = o_pool.tile([P, NJ, H], f32,
                                       name=f"ob{rep}_{m}", tag="ob")
                    if do_g2:
                        last = m == NM - 1
                        for j in range(NJ):
                            g2_group(ht, obig, j)
                            if last and "outdma" not in skip:
                                # drain the final macro per j-block so the
                                # kernel doesn't end on a monolithic store
                                out_dma.dma_start(
                                    out[r + j * P:r + (j + 1) * P, :],
                                    obig[:, j, :])
                        if not last and "outdma" not in skip:
                            out_dma.dma_start(
                                out[r:r + T, :].rearrange(
                                    "(j p) h -> p j h", p=P),
                                obig[:])
                    ht = ht_next

            with reps_ctx:
                for rep in range(unroll):
                    emit_body(rep)

    nc.compile()
    return nc


def kernel(inputs, w1, b1, w2, b2):
    from concourse.bass_utils import run_bass_kernel_spmd

    inputs = np.asarray(inputs, dtype=np.float32)
    w1 = np.asarray(w1, dtype=np.float32)
    b1 = np.asarray(b1, dtype=np.float32)
    w2 = np.asarray(w2, dtype=np.float32)
    b2 = np.asarray(b2, dtype=np.float32)

    B, E, C, H = inputs.shape
    tok = B * C
    # [B, E, C, H] -> per-expert TRANSPOSED token matrix [E, H, B*C]
    xT = np.ascontiguousarray(
        inputs.transpose(1, 3, 0, 2).reshape(E, H, tok))
    if _MM_MODE == "bf16":
        import ml_dtypes
        bf16 = ml_dtypes.bfloat16
        xT, w1, w2 = xT.astype(bf16), w1.astype(bf16), w2.astype(bf16)

    if "nc" not in _cache:
        _cache["nc"] = build_nc()
    nc = _cache["nc"]

    in_maps = [
        {
            "x": xT[e],
            "w1": np.ascontiguousarray(w1[e]),
            "b1": np.ascontiguousarray(b1[e]),
            "w2": np.ascontiguousarray(w2[e]),
            "b2": np.ascontiguousarray(b2[e]),
        }
        for e in range(E)
    ]
    res = run_bass_kernel_spmd(nc, in_maps, core_ids=list(range(E)))
    o = np.stack([res.results[e]["out"] for e in range(E)])  # [E, tok, H]
    return np.ascontiguousarray(
        o.reshape(E, B, C, H).transpose(1, 0, 2, 3))


# revision 15
# speedup vs baseline: 1.1878x; 1.1878x over previous
"""Expert-parallel MoE FFN kernel for Trainium2 (8 NeuronCores).

Problem: inputs [B=2, E=8, C=8192, H=512], per-expert FFN
    h   = gelu_tanh(x_e @ w1_e + b1_e)        (w1: [E, H, F=2048])
    out = h @ w2_e + b2_e                     (w2: [E, F, H])

Sharding: expert-parallel — core e owns expert e's tokens [B*C, H] and
weights; no cross-core communication.

Per-core dataflow (matmuls in bf16, fp32 PSUM accumulation):
    x is transposed to xT [H, tok] and cast to bf16 on the HOST, so each
    macro tile DMA-loads straight into the GEMM1 rhs layout — no PE
    transposes at all (they cost ~109ns each = 56us/core in the fp32r
    ancestor of this kernel).
    GEMM1: hT[f,:] += w1[hk, f-chunk].T @ xT[hk, :]   (lhsT = w1, natural layout)
    gelu on ACT during PSUM->SBUF eviction (bias = b1 per-partition)
    GEMM2: out[tok,:] += hT[fk, tok-chunk].T @ w2[fk, :] (lhsT = hT, rhs = w2 natural)
    b2 add on DVE during PSUM->SBUF eviction

This is PE-array-roofline-bound: 4096 matmuls x [128k x 128m x 512n]
per core stream at ~1 column/cycle @ 2.4 GHz warm = ~874us; bf16 beats
fp32r only via fast-weight-load (~-13ns/MM) and faster DMA, not FLOPs.
The measured MM stream runs ~220ns/MM at >99% PE occupancy.

Scheduling: x loads prefetched two macro tiles ahead on the sync
(HWDGE) queue; output stores ride the gpsimd (SWDGE) queue so an input
load never queues behind an output store; GEMM1(m+1) is emitted before
GEMM2(m) so gelu evictions complete under GEMM2's PE window; GEMM1
PSUM evictions alternate ACT/DVE engines (back-to-back ACT PSUM reads
collapse PE throughput ~17x); ~40 dummy matmuls at program start warm
the HAM clock gate (1.2 -> 2.4 GHz) while the head DMAs fly; only w1's
first f-slice gates the first matmul — the rest of the weights load on
the scalar HWDGE ring behind it.

Accuracy: bf16 inputs/weights with fp32 accumulation give rel err
~3.4e-3 vs the fp32 reference (gate: 2e-2). Biases stay fp32; the
output is fp32.
"""

import numpy as np

_B, _E, _C, _H, _F = 2, 8, 8192, 512, 2048
_TOK = _B * _C  # 16384 tokens per expert
_P = 128
_T = 512  # tokens per macro tile

# "bf16" halves nothing on paper (cost model says 1 cyc/row either way)
# but measures materially faster on HW thanks to FWL weight loads; the
# end-to-end rel err vs the fp32 reference is ~3.4e-3 (numpy estimate),
# well inside the 2e-2 gate.
_MM_MODE = "bf16"  # "bf16" | "f32r"

_cache = {}


def build_nc(tok=_TOK, act_name="Gelu_apprx_tanh", n_devices=_E,
             loop_reps=1, skip=(), psum_cfg=(6, 2), prefetch=2,
             out_q="gpsimd", mm_mode=_MM_MODE, unroll=1):
    import contextlib

    import concourse.mybir as mybir
    import concourse.tile as tile
    from concourse import bacc

    H, F, P, T = _H, _F, _P, _T
    HK, FK = H // P, F // P  # 4, 16 contraction chunks
    NJ = T // P  # 4 token sub-blocks per macro tile
    NM = tok // T  # macro tiles
    f32 = mybir.dt.float32
    f32r = mybir.dt.float32r
    mmdt = mybir.dt.bfloat16 if mm_mode == "bf16" else f32r
    act = getattr(mybir.ActivationFunctionType, act_name)

    nc = bacc.Bacc("TRN2", debug=False, target_bir_lowering=False,
                   num_devices=n_devices)
    # x arrives pre-transposed (and pre-cast in bf16 mode) from the host
    x = nc.dram_tensor("x", [H, tok], mmdt, kind="ExternalInput").ap()
    w1 = nc.dram_tensor("w1", [H, F], mmdt, kind="ExternalInput").ap()
    b1 = nc.dram_tensor("b1", [F], f32, kind="ExternalInput").ap()
    w2 = nc.dram_tensor("w2", [F, H], mmdt, kind="ExternalInput").ap()
    b2 = nc.dram_tensor("b2", [H], f32, kind="ExternalInput").ap()
    out = nc.dram_tensor("out", [tok, H], f32, kind="ExternalOutput").ap()

    with tile.TileContext(nc) as tc:
        with (
            tc.tile_pool(name="const", bufs=1) as const,
            tc.tile_pool(name="xt", bufs=1 + prefetch) as xt_pool,
            tc.tile_pool(name="ht", bufs=2) as ht_pool,
            tc.tile_pool(name="obuf", bufs=2) as o_pool,
            tc.tile_pool(name="scr", bufs=4) as scratch,
            tc.tile_pool(name="ps1", bufs=psum_cfg[0], space="PSUM") as ps1,
            tc.tile_pool(name="ps2", bufs=psum_cfg[1], space="PSUM") as ps2,
        ):
            # --- weights / constants, resident in SBUF for the whole kernel.
            # Critical path to the first GEMM1 matmul is w1's fm=0 slice +
            # xt(0); only that slice rides the sync queue ahead of the x
            # prefetches. Everything else (b-vectors, the rest of w1, w2)
            # rides the scalar engine's HWDGE ring, which GEMM1 overtakes
            # only after several macro tiles.
            w1_sb = const.tile([P, HK, F], mmdt)
            w2_sb = const.tile([P, FK, H], mmdt)
            nc.sync.dma_start(
                w1_sb[:, :, 0:P],
                w1[:, 0:P].rearrange("(hk p) f -> p hk f", p=P))
            b1_sb = const.tile([P, FK], f32)
            nc.scalar.dma_start(b1_sb[:], b1.rearrange("(fk p) -> p fk", p=P))
            b2_row = const.tile([1, H], f32)
            nc.scalar.dma_start(b2_row[:], b2[None, :])
            WC = (F - P) // 3
            for c in range(3):
                fs = P + c * WC
                nc.scalar.dma_start(
                    w1_sb[:, :, fs:fs + WC],
                    w1[:, fs:fs + WC].rearrange("(hk p) f -> p hk f", p=P))
            nc.scalar.dma_start(
                w2_sb[:], w2.rearrange("(fk p) h -> p fk h", p=P))
            ones = const.tile([1, P], f32)
            nc.any.memset(ones[:], 1.0)
            # ~40 dummy matmuls warm the PE HAM clock gate (cold = 1.2 GHz,
            # warm = 2.4 GHz after ~3.4us of activity) while the head DMAs
            # are still in flight, so the real GEMM stream starts warm.
            ps_warm = ps1.tile([P, T], f32, tag="ph", name="warm")
            for wi in range(40):
                nc.tensor.matmul(ps_warm[:, 0:P], ones[:], ones[:],
                                 start=(wi == 0), stop=(wi == 39))
            # broadcast b2 across all 128 partitions via a K=1 matmul
            b2_bc = const.tile([P, H], f32)
            ps_b2 = ps2.tile([P, H], f32, tag="po")
            nc.tensor.matmul(ps_b2[:], ones[:], b2_row[:], start=True, stop=True)
            nc.vector.tensor_copy(b2_bc[:], ps_b2[:])

            def load_x(key, m):
                """Prefetch macro tile m of xT into SBUF (GEMM1 rhs layout)."""
                xt = xt_pool.tile([P, HK, T], mmdt, name=f"xt{key}", tag="xt")
                if "indma" not in skip:
                    nc.sync.dma_start(
                        xt[:],
                        x[:, m * T:(m + 1) * T].rearrange(
                            "(hk p) t -> p hk t", p=P))
                return xt

            def g1_group(xt, ht, m, fm):
                """One GEMM1 accumulation group (4 matmuls) + gelu eviction.

                Evictions alternate engines: even fm goes ACT gelu straight
                from PSUM; odd fm is DVE-copied to SBUF scratch first and
                gelu'd from there. Back-to-back ACT PSUM evictions of
                consecutive groups collapse PE throughput ~17x (measured);
                alternating eliminates that.
                """
                ph = ps1.tile([P, T], f32, name=f"ph{m}_{fm}", tag="ph")
                for hk in range(HK):
                    nc.tensor.matmul(
                        ph[:],
                        w1_sb[:, hk, fm * P:(fm + 1) * P],
                        xt[:, hk, :],
                        start=(hk == 0),
                        stop=(hk == HK - 1),
                    )
                if fm % 2 == 0:
                    nc.scalar.activation(
                        ht[:, fm, :], ph[:], act, bias=b1_sb[:, fm:fm + 1])
                else:
                    sc = scratch.tile([P, T], f32, name=f"sc{m}_{fm}", tag="sc")
                    nc.vector.tensor_copy(sc[:], ph[:])
                    nc.scalar.activation(
                        ht[:, fm, :], sc[:], act, bias=b1_sb[:, fm:fm + 1])

            def g2_group(ht, obig, j):
                """One GEMM2 accumulation group (16 matmuls) + b2 eviction."""
                po = ps2.tile([P, H], f32, tag="po", name="po")
                for fk in range(FK):
                    nc.tensor.matmul(
                        po[:],
                        ht[:, fk, j * P:(j + 1) * P],
                        w2_sb[:, fk, :],
                        start=(fk == 0),
                        stop=(fk == FK - 1),
                    )
                nc.vector.tensor_add(obig[:, j, :], po[:], b2_bc[:])

            def new_ht(key):
                return ht_pool.tile([P, FK, T], mmdt,
                                    name=f"ht{key}", tag="ht")

            out_dma = {"gpsimd": nc.gpsimd, "sync": nc.sync,
                       "scalar": nc.scalar}[out_q]

            # loop_reps > 1 wraps the body in a hardware loop — used only by
            # the timing harness to amortize per-dispatch overhead. For_i
            # inserts an all-engine barrier per iteration, so `unroll`
            # repeats the body within one iteration to keep the measured
            # slope closer to the barrier-free steady state.
            assert loop_reps % unroll == 0
            n_iters = loop_reps // unroll
            reps_ctx = (tc.For_i(0, n_iters, 1) if n_iters > 1
                        else contextlib.nullcontext())
            # Software pipeline, one macro ahead on GEMM1, `prefetch` macros
            # ahead on the x DMA:
            #   [dma x(m+2)] ; G1(m+1) x16 ; G2(m) x4 ; out(m)
            # GEMM1(m+1) runs before GEMM2(m) so ACT gelu evictions of ht(m+1)
            # complete under GEMM2(m)'s PE window.
            do_g1 = "gemm1" not in skip
            do_g2 = "gemm2" not in skip

            def emit_body(rep):
                xts = {m: load_x(f"{rep}_{m}", m)
                       for m in range(min(prefetch, NM))}
                ht = new_ht(f"{rep}_0")
                if do_g1:
                    for fm in range(FK):
                        g1_group(xts[0], ht, f"{rep}_0", fm)
                for m in range(NM):
                    r = m * T
                    if m + prefetch < NM:
                        xts[m + prefetch] = load_x(
                            f"{rep}_{m + prefetch}", m + prefetch)
                    if m + 1 < NM:
                        ht_next = new_ht(f"{rep}_{m + 1}")
                        if do_g1:
                            for fm in range(FK):
                                g1_group(xts[m + 1], ht_next,
                                         f"{rep}_{m + 1}", fm)
                    else:
                        ht_next = None
                    xts.pop(m, None)
                    obig = o_pool.tile([P, NJ, H], f32,
                                       name=f"ob{rep}_{m}", tag="ob")
                    if do_g2:
                        last = m == NM - 1
                        for j in range(NJ):
                            g2_group(ht, obig, j)
                            if last and "outdma" not in skip:
                                # drain the final macro per j-block so the
                                # kernel doesn't end on a monolithic store
                                out_dma.dma_start(
                                    out[r + j * P:r + (j + 1) * P, :],
                                    obig[:, j, :])
                        if not last and "outdma" not in skip:
                            out_dma.dma_start(
                                out[r:r + T, :].rearrange(
                                    "(j p) h -> p j h", p=P),
                                obig[:])
                    ht = ht_next

            with reps_ctx:
                for rep in range(unroll):
                    emit_body(rep)

    nc.compile()
    return nc


def kernel(inputs, w1, b1, w2, b2):
    from concourse.bass_utils import run_bass_kernel_spmd

    inputs = np.asarray(inputs, dtype=np.float32)
    w1 = np.asarray(w1, dtype=np.float32)
    b1 = np.asarray(b1, dtype=np.float32)
    w2 = np.asarray(w2, dtype=np.float32)
    b2 = np.asarray(b2, dtype=np.float32)

    B, E, C, H = inputs.shape
    tok = B * C
    # [B, E, C, H] -> per-expert TRANSPOSED token matrix [E, H, B*C]
    xT = np.ascontiguousarray(
        inputs.transpose(1, 3, 0, 2).reshape(E, H, tok))
    if _MM_MODE == "bf16":
        import ml_dtypes
        bf16 = ml_dtypes.bfloat16
        xT, w1, w2 = xT.astype(bf16), w1.astype(bf16), w2.astype(bf16)

    if "nc" not in _cache:
        _cache["nc"] = build_nc()
    nc = _cache["nc"]

    in_maps = [
        {
            "x": xT[e],
            "w1": np.ascontiguousarray(w1[e]),
            "b1": np.ascontiguousarray(b1[e]),
            "w2": np.ascontiguousarray(w2[e]),
            "b2": np.ascontiguousarray(b2[e]),
        }
        for e in range(E)
    ]
    res = run_bass_kernel_spmd(nc, in_maps, core_ids=list(range(E)))
    o = np.stack([res.results[e]["out"] for e in range(E)])  # [E, tok, H]
    return np.ascontiguousarray(
        o.reshape(E, B, C, H).transpose(1, 0, 2, 3))


# revision 16
# speedup vs baseline: 1.1964x; 1.0073x over previous
"""Expert-parallel MoE FFN kernel for Trainium2 (8 NeuronCores).

Problem: inputs [B=2, E=8, C=8192, H=512], per-expert FFN
    h   = gelu_tanh(x_e @ w1_e + b1_e)        (w1: [E, H, F=2048])
    out = h @ w2_e + b2_e                     (w2: [E, F, H])

Sharding: expert-parallel — core e owns expert e's tokens [B*C, H] and
weights; no cross-core communication.

Per-core dataflow (matmuls in bf16, fp32 PSUM accumulation):
    x is transposed to xT [H, tok] and cast to bf16 on the HOST, so each
    macro tile DMA-loads straight into the GEMM1 rhs layout — no PE
    transposes at all (they cost ~109ns each = 56us/core in the fp32r
    ancestor of this kernel).
    GEMM1: hT[f,:] += w1[hk, f-chunk].T @ xT[hk, :]   (lhsT = w1, natural layout)
    gelu on ACT during PSUM->SBUF eviction (bias = b1 per-partition)
    GEMM2: out[tok,:] += hT[fk, tok-chunk].T @ w2[fk, :] (lhsT = hT, rhs = w2 natural)
    b2 add on DVE during PSUM->SBUF eviction

This is PE-array-roofline-bound: 4096 matmuls x [128k x 128m x 512n]
per core stream at ~1 column/cycle @ 2.4 GHz warm = ~874us; bf16 beats
fp32r only via fast-weight-load (~-13ns/MM) and faster DMA, not FLOPs.
The measured MM stream runs ~220ns/MM at >99% PE occupancy.

Scheduling: x loads prefetched two macro tiles ahead on the sync
(HWDGE) queue; output stores ride the gpsimd (SWDGE) queue so an input
load never queues behind an output store; GEMM1(m+1) is emitted before
GEMM2(m) so gelu evictions complete under GEMM2's PE window; GEMM1
PSUM evictions alternate ACT/DVE engines (back-to-back ACT PSUM reads
collapse PE throughput ~17x); ~40 dummy matmuls at program start warm
the HAM clock gate (1.2 -> 2.4 GHz) while the head DMAs fly; only w1's
first f-slice gates the first matmul — the rest of the weights load on
the scalar HWDGE ring behind it.

Accuracy: bf16 inputs/weights with fp32 accumulation give rel err
~3.4e-3 vs the fp32 reference (gate: 2e-2). Biases stay fp32; the
output is fp32.
"""

import numpy as np

_B, _E, _C, _H, _F = 2, 8, 8192, 512, 2048
_TOK = _B * _C  # 16384 tokens per expert
_P = 128
_T = 512  # tokens per macro tile

# "bf16" halves nothing on paper (cost model says 1 cyc/row either way)
# but measures materially faster on HW thanks to FWL weight loads; the
# end-to-end rel err vs the fp32 reference is ~3.4e-3 (numpy estimate),
# well inside the 2e-2 gate.
_MM_MODE = "bf16"  # "bf16" | "f32r"

_cache = {}


def build_nc(tok=_TOK, act_name="Gelu_apprx_tanh", n_devices=_E,
             loop_reps=1, skip=(), psum_cfg=(6, 2), prefetch=2,
             out_q="gpsimd", mm_mode=_MM_MODE, unroll=1):
    import contextlib

    import concourse.mybir as mybir
    import concourse.tile as tile
    from concourse import bacc

    H, F, P, T = _H, _F, _P, _T
    HK, FK = H // P, F // P  # 4, 16 contraction chunks
    NJ = T // P  # 4 token sub-blocks per macro tile
    NM = tok // T  # macro tiles
    f32 = mybir.dt.float32
    f32r = mybir.dt.float32r
    mmdt = mybir.dt.bfloat16 if mm_mode == "bf16" else f32r
    act = getattr(mybir.ActivationFunctionType, act_name)

    nc = bacc.Bacc("TRN2", debug=False, target_bir_lowering=False,
                   num_devices=n_devices)
    # x arrives pre-transposed (and pre-cast in bf16 mode) from the host
    x = nc.dram_tensor("x", [H, tok], mmdt, kind="ExternalInput").ap()
    w1 = nc.dram_tensor("w1", [H, F], mmdt, kind="ExternalInput").ap()
    b1 = nc.dram_tensor("b1", [F], f32, kind="ExternalInput").ap()
    w2 = nc.dram_tensor("w2", [F, H], mmdt, kind="ExternalInput").ap()
    b2 = nc.dram_tensor("b2", [H], f32, kind="ExternalInput").ap()
    out = nc.dram_tensor("out", [tok, H], f32, kind="ExternalOutput").ap()

    with tile.TileContext(nc) as tc:
        with (
            tc.tile_pool(name="const", bufs=1) as const,
            tc.tile_pool(name="xt", bufs=1 + prefetch) as xt_pool,
            tc.tile_pool(name="ht", bufs=2) as ht_pool,
            tc.tile_pool(name="obuf", bufs=2) as o_pool,
            tc.tile_pool(name="scr", bufs=4) as scratch,
            tc.tile_pool(name="ps1", bufs=psum_cfg[0], space="PSUM") as ps1,
            tc.tile_pool(name="ps2", bufs=psum_cfg[1], space="PSUM") as ps2,
        ):
            # --- weights / constants, resident in SBUF for the whole kernel.
            # Critical path to the first GEMM1 matmul is w1's fm=0 slice +
            # xt(0); only that slice rides the sync queue ahead of the x
            # prefetches. Everything else (b-vectors, the rest of w1, w2)
            # rides the scalar engine's HWDGE ring, which GEMM1 overtakes
            # only after several macro tiles.
            w1_sb = const.tile([P, HK, F], mmdt)
            w2_sb = const.tile([P, FK, H], mmdt)
            nc.sync.dma_start(
                w1_sb[:, :, 0:P],
                w1[:, 0:P].rearrange("(hk p) f -> p hk f", p=P))
            b1_sb = const.tile([P, FK], f32)
            nc.scalar.dma_start(b1_sb[:], b1.rearrange("(fk p) -> p fk", p=P))
            b2_row = const.tile([1, H], f32)
            nc.scalar.dma_start(b2_row[:], b2[None, :])
            # w1-rest and w2 ride the (otherwise idle at head) gpsimd
            # SWDGE queue: issuing them from the scalar queue would block
            # the first GEMM1 macro's gelu evictions behind the DMA
            # issues and stall the PE once ps1 fills.
            WC = (F - P) // 3
            for c in range(3):
                fs = P + c * WC
                nc.gpsimd.dma_start(
                    w1_sb[:, :, fs:fs + WC],
                    w1[:, fs:fs + WC].rearrange("(hk p) f -> p hk f", p=P))
            nc.gpsimd.dma_start(
                w2_sb[:], w2.rearrange("(fk p) h -> p fk h", p=P))
            ones = const.tile([1, P], f32)
            nc.any.memset(ones[:], 1.0)
            ones_mm = const.tile([1, P], mmdt)
            nc.any.memset(ones_mm[:], 1.0)
            # ~40 dummy matmuls warm the PE HAM clock gate (cold = 1.2 GHz,
            # warm = 2.4 GHz after ~3.4us of activity) while the head DMAs
            # are still in flight, so the real GEMM stream starts warm.
            # bf16 operands: an f32 warmup runs 4 cyc/row and overshoots
            # the DMA head, delaying the first real matmul.
            ps_warm = ps1.tile([P, T], f32, tag="ph", name="warm")
            for wi in range(40):
                nc.tensor.matmul(ps_warm[:, 0:P], ones_mm[:], ones_mm[:],
                                 start=(wi == 0), stop=(wi == 39))
            # broadcast b2 across all 128 partitions via a K=1 matmul
            b2_bc = const.tile([P, H], f32)
            ps_b2 = ps2.tile([P, H], f32, tag="po")
            nc.tensor.matmul(ps_b2[:], ones[:], b2_row[:], start=True, stop=True)
            nc.vector.tensor_copy(b2_bc[:], ps_b2[:])

            def load_x(key, m):
                """Prefetch macro tile m of xT into SBUF (GEMM1 rhs layout)."""
                xt = xt_pool.tile([P, HK, T], mmdt, name=f"xt{key}", tag="xt")
                if "indma" not in skip:
                    nc.sync.dma_start(
                        xt[:],
                        x[:, m * T:(m + 1) * T].rearrange(
                            "(hk p) t -> p hk t", p=P))
                return xt

            def g1_group(xt, ht, m, fm):
                """One GEMM1 accumulation group (4 matmuls) + gelu eviction.

                Evictions alternate engines: even fm goes ACT gelu straight
                from PSUM; odd fm is DVE-copied to SBUF scratch first and
                gelu'd from there. Back-to-back ACT PSUM evictions of
                consecutive groups collapse PE throughput ~17x (measured);
                alternating eliminates that.
                """
                ph = ps1.tile([P, T], f32, name=f"ph{m}_{fm}", tag="ph")
                for hk in range(HK):
                    nc.tensor.matmul(
                        ph[:],
                        w1_sb[:, hk, fm * P:(fm + 1) * P],
                        xt[:, hk, :],
                        start=(hk == 0),
                        stop=(hk == HK - 1),
                    )
                if fm % 2 == 0:
                    nc.scalar.activation(
                        ht[:, fm, :], ph[:], act, bias=b1_sb[:, fm:fm + 1])
                else:
                    sc = scratch.tile([P, T], f32, name=f"sc{m}_{fm}", tag="sc")
                    nc.vector.tensor_copy(sc[:], ph[:])
                    nc.scalar.activation(
                        ht[:, fm, :], sc[:], act, bias=b1_sb[:, fm:fm + 1])

            def g2_group(ht, obig, j):
                """One GEMM2 accumulation group (16 matmuls) + b2 eviction."""
                po = ps2.tile([P, H], f32, tag="po", name="po")
                for fk in range(FK):
                    nc.tensor.matmul(
                        po[:],
                        ht[:, fk, j * P:(j + 1) * P],
                        w2_sb[:, fk, :],
                        start=(fk == 0),
                        stop=(fk == FK - 1),
                    )
                nc.vector.tensor_add(obig[:, j, :], po[:], b2_bc[:])

            def new_ht(key):
                return ht_pool.tile([P, FK, T], mmdt,
                                    name=f"ht{key}", tag="ht")

            out_dma = {"gpsimd": nc.gpsimd, "sync": nc.sync,
                       "scalar": nc.scalar}[out_q]

            # loop_reps > 1 wraps the body in a hardware loop — used only by
            # the timing harness to amortize per-dispatch overhead. For_i
            # inserts an all-engine barrier per iteration, so `unroll`
            # repeats the body within one iteration to keep the measured
            # slope closer to the barrier-free steady state.
            assert loop_reps % unroll == 0
            n_iters = loop_reps // unroll
            reps_ctx = (tc.For_i(0, n_iters, 1) if n_iters > 1
                        else contextlib.nullcontext())
            # Software pipeline, one macro ahead on GEMM1, `prefetch` macros
            # ahead on the x DMA:
            #   [dma x(m+2)] ; G1(m+1) x16 ; G2(m) x4 ; out(m)
            # GEMM1(m+1) runs before GEMM2(m) so ACT gelu evictions of ht(m+1)
            # complete under GEMM2(m)'s PE window.
            do_g1 = "gemm1" not in skip
            do_g2 = "gemm2" not in skip

            def emit_body(rep):
                xts = {m: load_x(f"{rep}_{m}", m)
                       for m in range(min(prefetch, NM))}
                ht = new_ht(f"{rep}_0")
                if do_g1:
                    for fm in range(FK):
                        g1_group(xts[0], ht, f"{rep}_0", fm)
                for m in range(NM):
                    r = m * T
                    if m + prefetch < NM:
                        xts[m + prefetch] = load_x(
                            f"{rep}_{m + prefetch}", m + prefetch)
                    if m + 1 < NM:
                        ht_next = new_ht(f"{rep}_{m + 1}")
                        if do_g1:
                            for fm in range(FK):
                                g1_group(xts[m + 1], ht_next,
                                         f"{rep}_{m + 1}", fm)
                    else:
                        ht_next = None
                    xts.pop(m, None)
                    obig = o_pool.tile([P, NJ, H], f32,
                                       name=f"ob{rep}_{m}", tag="ob")
                    if do_g2:
                        last = m == NM - 1
                        for j in range(NJ):
                            g2_group(ht, obig, j)
                            if last and "outdma" not in skip:
                                # drain the final macro per j-block so the
                                # kernel doesn't end on a monolithic store
                                out_dma.dma_start(
                                    out[r + j * P:r + (j + 1) * P, :],
                                    obig[:, j, :])
                        if not last and "outdma" not in skip:
                            out_dma.dma_start(
                                out[r:r + T, :].rearrange(
                                    "(j p) h -> p j h", p=P),
                                obig[:])
                    ht = ht_next

            with reps_ctx:
                for rep in range(unroll):
                    emit_body(rep)

    nc.compile()
    return nc


def kernel(inputs, w1, b1, w2, b2):
    from concourse.bass_utils import run_bass_kernel_spmd

    inputs = np.asarray(inputs, dtype=np.float32)
    w1 = np.asarray(w1, dtype=np.float32)
    b1 = np.asarray(b1, dtype=np.float32)
    w2 = np.asarray(w2, dtype=np.float32)
    b2 = np.asarray(b2, dtype=np.float32)

    B, E, C, H = inputs.shape
    tok = B * C
    # [B, E, C, H] -> per-expert TRANSPOSED token matrix [E, H, B*C]
    xT = np.ascontiguousarray(
        inputs.transpose(1, 3, 0, 2).reshape(E, H, tok))
    if _MM_MODE == "bf16":
        import ml_dtypes
        bf16 = ml_dtypes.bfloat16
        xT, w1, w2 = xT.astype(bf16), w1.astype(bf16), w2.astype(bf16)

    if "nc" not in _cache:
        _cache["nc"] = build_nc()
    nc = _cache["nc"]

    in_maps = [
        {
            "x": xT[e],
            "w1": np.ascontiguousarray(w1[e]),
            "b1": np.ascontiguousarray(b1[e]),
            "w2": np.ascontiguousarray(w2[e]),
            "b2": np.ascontiguousarray(b2[e]),
        }
        for e in range(E)
    ]
    res = run_bass_kernel_spmd(nc, in_maps, core_ids=list(range(E)))
    o = np.stack([res.results[e]["out"] for e in range(E)])  # [E, tok, H]
    return np.ascontiguousarray(
        o.reshape(E, B, C, H).transpose(1, 0, 2, 3))
